# revision 38
# baseline (speedup 1.0000x reference)
"""DualReprogrammingLayer Trainium2 kernel.

Sharding: 2 row-groups (B*L split in halves) x 4 head-groups (4 heads each).
Each core computes, for its 2048 rows and 4 heads (per block in {trend, detail}):
  KT = (Wk.T @ protoT)           (heads-slice, S)        [K-proj, f32r in, bf16 out]
  V  = (protoT.T @ Wv) + bv      (S, heads-slice)        [V-proj, f32r]
  qT = (Wq.T @ xT) + bq          (heads-slice, rows)     [bf16]
  scoresT = KT_h @ qT_h          (S, rows) per head      [bf16, 2-head row-packed]
  P  = exp(scoresT / 8)                                  [ACT, f32r out]
  A_ext = [V_h | ones].T @ P     (64+64, rows)           [f32r; rows 64:128 = denom]
  gate = sigmoid(relu(cat @ W1) @ W2)  (on-device, bf16)
  A_scaled = A * (gate_coef / denom)                     [bf16]
  out_partial = [A_t; A_d].T-stack @ [Wo_t; Wo_d]        [bf16]
Host sums the 4 head-group partials per row-group.
"""
import sys
sys.path.insert(0, '/opt/trn_rl_repo')
from contextlib import ExitStack

import numpy as np
import ml_dtypes

import concourse.bass as bass
import concourse.tile as tile
from concourse import bacc, mybir

F32 = mybir.dt.float32
F32R = mybir.dt.float32r
BF16 = mybir.dt.bfloat16
AF = mybir.ActivationFunctionType
bf16 = ml_dtypes.bfloat16

B, L, D, S, DLLM, H, E = 4, 1024, 1024, 1000, 4096, 16, 64
RG, HG = 2, 4                 # row-groups x head-groups = 8 cores
R = (B * L) // RG             # 2048 rows per core
NH = H // HG                  # 4 heads per core
HEC = NH * E                  # 256
SCH, NSC = 125, 8             # S = 8 chunks of 125
RC, NRC = 512, 4              # rows = 4 chunks of 512
KD = D // 128                 # 8 k-chunks for d_model
KL = DLLM // 128              # 32 k-chunks for d_llm

_CACHE = {}
LAST_RESULTS = None           # set by kernel(): BassKernelResults


def _build(with_bo):
    nc = bacc.Bacc("TRN2", target_bir_lowering=False, debug=False)

    def din(name, shape, dt):
        return nc.dram_tensor(name, list(shape), dt, kind="ExternalInput")

    xT = {b: din(f"xT_{b}", (D, R), BF16) for b in "td"}
    pT = {b: din(f"pT_{b}", (DLLM, S), BF16) for b in "td"}
    wq = {b: din(f"wq_{b}", (D, HEC), BF16) for b in "td"}
    wk = {b: din(f"wk_{b}", (DLLM, HEC), BF16) for b in "td"}
    wv = {b: din(f"wv_{b}", (DLLM, HEC), BF16) for b in "td"}
    wo = din("wo", (2 * HEC, DLLM), BF16)            # [t rows | d rows]
    w1 = din("w1", (2 * D, D), BF16)
    w2 = din("w2", (D, 1), BF16)
    bq2 = din("bq2", (128, 4), F32)                  # cols: t-mc0, t-mc1, d-mc0, d-mc1
    bk2 = din("bk2", (128, 4), F32)
    bvv = din("bv", (1, 2 * HEC), BF16)               # [t 256 | d 256]
    gb1 = din("gb1", (128, KD), F32)
    gb2 = din("gb2", (1, 1), F32)
    ones_d = din("ones", (1, 2048), F32)
    bo2 = din("bo2", (2, DLLM), BF16) if with_bo else None
    out = nc.dram_tensor("out", [R, DLLM], F32, kind="ExternalOutput")

    with tile.TileContext(nc) as tc, ExitStack() as ctx:
        # ---- persistent pools (live across phases) ----
        pers = ctx.enter_context(tc.tile_pool(name="pers", bufs=1))
        kt_sb = {}    # block -> tile (128, 2, S) bf16 : HE chunk mc at [:, mc, :]
        vx_sb = {}    # block -> tile (125, NSC, NH, 128) f32r : [V_h | ones]
        qt_sb = {}    # block -> tile (128, 2, R) bf16
        for b in "td":
            kt_sb[b] = pers.tile([128, 2, S], BF16, tag=f"kt_{b}", name=f"kt_{b}")
            vx_sb[b] = pers.tile([SCH, NSC, NH, 65], BF16, tag=f"vx_{b}", name=f"vx_{b}")
            qt_sb[b] = pers.tile([128, 2, R], BF16, tag=f"qt_{b}", name=f"qt_{b}")
        gate_sb = pers.tile([1, R], F32, tag="gate")     # sigmoid output
        omg_sb = pers.tile([1, R], F32, tag="omg")       # 1 - gate
        ones125 = pers.tile([1, SCH], BF16, tag="ones125")
        nc.vector.memset(ones125[:], 1.0)
        onesrow = pers.tile([1, RC], F32, tag="onesrow")
        nc.vector.memset(onesrow[:], 1.0)
        bq_sb = pers.tile([128, 4], F32, tag="bq")
        nc.sync.dma_start(bq_sb[:], bq2.ap())
        bk_sb = pers.tile([128, 4], F32, tag="bk")
        nc.sync.dma_start(bk_sb[:], bk2.ap())
        bv_sb = pers.tile([1, 2 * HEC], BF16, tag="bv")
        nc.sync.dma_start(bv_sb[:], bvv.ap())
        gb1_sb = pers.tile([128, KD], F32, tag="gb1")
        nc.sync.dma_start(gb1_sb[:], gb1.ap())
        gb2_sb = pers.tile([1, 1], F32, tag="gb2")
        nc.sync.dma_start(gb2_sb[:], gb2.ap())
        if with_bo:
            bo_sb = pers.tile([2, DLLM], BF16, tag="bo")
            nc.sync.dma_start(bo_sb[:], bo2.ap())
            g2_sb = pers.tile([2, R], BF16, tag="g2")

        # ---- prefetch pools: weights for later phases, loaded during phase A.
        # p_pre2 (wo) lives through phase C; p_pre1 (W1/wq) releases after B.
        p_pre2 = ctx.enter_context(tc.tile_pool(name="p_pre2", bufs=1))
        wo_t = p_pre2.tile([128, 4, DLLM], BF16, tag="wo")
        pre1ctx = ExitStack()
        p_pre1 = pre1ctx.enter_context(tc.tile_pool(name="p_pre1", bufs=1))
        w1_t = p_pre1.tile([128, 2 * KD, D], BF16, tag="w1")
        w2_t = p_pre1.tile([128, KD, 1], BF16, tag="w2")
        wq_t = {}
        for b in "td":
            wq_t[b] = p_pre1.tile([128, KD, HEC], BF16, tag=f"wq_{b}", name=f"wq_{b}")

        def emit_prefetch():
            nc.scalar.dma_start(w1_t[:], w1.ap().rearrange("(c p) m -> p c m", p=128))
            nc.scalar.dma_start(w2_t[:], w2.ap().rearrange("(c p) m -> p c m", p=128))
            for b in "td":
                nc.scalar.dma_start(
                    wq_t[b][:], wq[b].ap().rearrange("(c p) m -> p c m", p=128))
            nc.scalar.dma_start(wo_t[:], wo.ap().rearrange("(c p) n -> p c n", p=128))

        # ---- phase A: K/V projections (proto and weights streamed per k-chunk) ----
        with ExitStack() as actx:
            p_pt = actx.enter_context(tc.tile_pool(name="p_pt", bufs=8))
            p_wc = actx.enter_context(tc.tile_pool(name="p_wc", bufs=1))
            psA = actx.enter_context(tc.tile_pool(name="psA", bufs=1, space="PSUM"))
            # PE warm-up: ~8us of dummy matmuls on memset tiles so the HAM
            # clock-gate reaches 8/8 before the first real matmul, overlapping
            # the initial DMA ramp. Results are discarded.
            wu_l = p_wc.tile([64, 128], BF16, tag="wu_l")
            nc.vector.memset(wu_l[:], 0.0)
            wu_r = p_wc.tile([64, 512], BF16, tag="wu_r")
            nc.vector.memset(wu_r[:], 0.0)

            for b in "td":
                vps = [psA.tile([SCH, 2, HEC], F32, tag=f"vps{i}", name=f"vps{i}")
                       for i in range(4)]
                kps = [psA.tile([128, 512], F32, tag=f"kps{i}", name=f"kps{i}")
                       for i in range(4)]
                if b == "t":
                    # discarded warm-up group in kps[0]'s bank, closed before
                    # the real accumulation group opens
                    for wi in range(36):
                        nc.tensor.matmul(kps[0][:], wu_l[:], wu_r[:],
                                         start=(wi == 0), stop=(wi == 35))
                wk_t = p_wc.tile([128, KL, HEC], BF16, tag="wk_t")
                wv_t = p_wc.tile([128, KL, HEC], BF16, tag="wv_t")
                wk_r4 = wk[b].ap().rearrange("(c p) m -> p c m", p=128)
                wv_r4 = wv[b].ap().rearrange("(c p) m -> p c m", p=128)
                pt_r = pT[b].ap().rearrange("(c p) s -> c p s", c=KL)
                for kc in range(KL):
                    if kc % (KL // 2) == 0:
                        hq = slice(kc, kc + KL // 2)
                        nc.scalar.dma_start(wv_t[:, hq, :], wv_r4[:, hq, :])
                        nc.scalar.dma_start(wk_t[:, hq, :], wk_r4[:, hq, :])
                    pt_t = p_pt.tile([128, S], BF16, tag="pt")
                    nc.sync.dma_start(pt_t[:], pt_r[kc])
                    wkc = wk_t[:, kc, :]
                    wvc = wv_t[:, kc, :]
                    for si in range(NSC):
                        # one accumulation group per PSUM bank: only the first
                        # half issues start=True (bank-wide clear covers both)
                        nc.tensor.matmul(
                            vps[si // 2][:, si % 2, :],
                            pt_t[:, si * SCH:(si + 1) * SCH],
                            wvc,
                            start=(kc == 0 and si % 2 == 0), stop=False)
                    for mc in range(2):
                        for ncc in range(2):
                            nc.tensor.matmul(
                                kps[mc * 2 + ncc][:, 0:500],
                                wk_t[:, kc, mc * 128:(mc + 1) * 128],
                                pt_t[:, ncc * 500:(ncc + 1) * 500],
                                start=(kc == 0), stop=(kc == KL - 1))
                boff = 0 if b == "t" else HEC
                for si in range(NSC):
                    nc.tensor.matmul(
                        vps[si // 2][:, si % 2, :],
                        ones125[:],
                        bv_sb[:, boff:boff + HEC],
                        start=False, stop=(si % 2 == 1))
                for si in range(NSC):
                    # copy V psum (125, 256) -> [:, si, :, 0:64] viewed as (125, 4, 64)
                    nc.vector.tensor_copy(
                        vx_sb[b][:, si, :, 0:64],
                        vps[si // 2][:, si % 2, :].rearrange("p (h e) -> p h e", h=NH))
                nc.vector.memset(vx_sb[b][:, :, :, 64:65], 1.0)
                for mc in range(2):
                    for ncc in range(2):
                        nc.scalar.activation(
                            kt_sb[b][:, mc, ncc * 500:(ncc + 1) * 500],
                            kps[mc * 2 + ncc][:, 0:500],
                            AF.Identity,
                            bias=bk_sb[:, (0 if b == "t" else 2) + mc:
                                       (0 if b == "t" else 2) + mc + 1])
                if b == "t":
                    emit_prefetch()

        # ---- phase B: gate + Q projections (per rows-chunk) ----
        with ExitStack() as bctx:
            p_x = bctx.enter_context(tc.tile_pool(name="p_x", bufs=3))
            p_h = bctx.enter_context(tc.tile_pool(name="p_h", bufs=2))
            psB = bctx.enter_context(tc.tile_pool(name="psB", bufs=2, space="PSUM"))
            for r in range(NRC - 1):
                rsl = slice(r * RC, (r + 1) * RC)
                xt = {}
                for b in "td":
                    xt[b] = p_x.tile([128, KD, RC], BF16, tag=f"x_{b}", name=f"x_{b}")
                    nc.sync.dma_start(
                        xt[b][:],
                        xT[b].ap().rearrange("(c p) n -> p c n", p=128)[:, :, rsl])
                # gate hidden: 8 m-chunks, contraction over 16 chunks (t then d)
                ht = p_h.tile([128, KD, RC], BF16, tag="ht")
                for mc in range(KD):
                    hps = psB.tile([128, RC], F32, tag="hps")
                    for kc in range(2 * KD):
                        nc.tensor.matmul(
                            hps[:],
                            w1_t[:, kc, mc * 128:(mc + 1) * 128],
                            xt["t" if kc < KD else "d"][:, kc % KD, :],
                            start=(kc == 0), stop=(kc == 2 * KD - 1))
                    nc.scalar.activation(
                        ht[:, mc, :], hps[:], AF.Relu,
                        bias=gb1_sb[:, mc:mc + 1])
                lps = psB.tile([1, RC], F32, tag="lps")
                for mc in range(KD):
                    nc.tensor.matmul(
                        lps[:], w2_t[:, mc, :], ht[:, mc, :],
                        start=(mc == 0), stop=(mc == KD - 1))
                nc.scalar.activation(
                    gate_sb[:, rsl], lps[:], AF.Sigmoid, bias=gb2_sb[:])
                nc.vector.tensor_sub(omg_sb[:, rsl], onesrow[:], gate_sb[:, rsl])
                if with_bo:
                    nc.vector.tensor_copy(g2_sb[0:1, rsl], gate_sb[:, rsl])
                    nc.vector.tensor_copy(g2_sb[1:2, rsl], omg_sb[:, rsl])
                # Q projections
                for b in "td":
                    for mc in range(2):
                        qps = psB.tile([128, RC], F32, tag="qps")
                        for kc in range(KD):
                            nc.tensor.matmul(
                                qps[:],
                                wq_t[b][:, kc, mc * 128:(mc + 1) * 128],
                                xt[b][:, kc, :],
                                start=(kc == 0), stop=(kc == KD - 1))
                        nc.scalar.activation(
                            qt_sb[b][:, mc, rsl], qps[:], AF.Identity,
                            bias=bq_sb[:, (0 if b == "t" else 2) + mc:
                                       (0 if b == "t" else 2) + mc + 1])
        # ---- phase C: attention + output projection ----
        # Software pipeline: QK+exp of unit u overlaps PV of unit u-1 at
        # s-chunk granularity; output-projection rows-blocks of the previous
        # rows-chunk interleave between units to keep the PE stream dense.
        with ExitStack() as cctx:
            p_p = cctx.enter_context(tc.tile_pool(name="p_p", bufs=4))
            p_a = cctx.enter_context(tc.tile_pool(name="p_a", bufs=2))
            p_s = cctx.enter_context(tc.tile_pool(name="p_s", bufs=2))
            p_o = cctx.enter_context(tc.tile_pool(name="p_o", bufs=2))
            psS = cctx.enter_context(tc.tile_pool(name="psS", bufs=1, space="PSUM"))
            psPV = cctx.enter_context(tc.tile_pool(name="psPV", bufs=1, space="PSUM"))
            psO = cctx.enter_context(tc.tile_pool(name="psO", bufs=2, space="PSUM"))

            def emit_qk_exp(b, mc, si, rsl):
                sps2 = psS.tile([SCH, 2, RC], F32, tag="sps", name="sps")
                for hh in range(2):  # row-packed pair, adjacent emission
                    po = hh * 64
                    nc.tensor.matmul(
                        sps2[:, hh, :],
                        kt_sb[b][po:po + 64, mc, si * SCH:(si + 1) * SCH],
                        qt_sb[b][po:po + 64, mc, rsl],
                        start=True, stop=True,
                        tile_position=(po, 0))
                p2 = p_p.tile([SCH, 2, RC], BF16, tag=f"p{si % 2}",
                              name=f"p{si % 2}")
                nc.scalar.activation(p2[:], sps2[:], AF.Exp, scale=0.125)
                return p2

            def emit_pv(aps, b, mc, si, p2):
                for hh in range(2):
                    h = mc * 2 + hh
                    nc.tensor.matmul(
                        aps[hh][:], vx_sb[b][:, si, h, :], p2[:, hh, :],
                        start=(si == 0), stop=(si == NSC - 1))

            def emit_norm(aps, b, mc, a2, rsl):
                gcoef = gate_sb if b == "t" else omg_sb
                for hh in range(2):
                    den1 = p_s.tile([1, RC], F32, tag="den1")
                    nc.vector.tensor_copy(den1[:], aps[hh][64:65, :])
                    rec1 = p_s.tile([1, RC], F32, tag="rec1")
                    nc.vector.reciprocal_approx_fast(rec1[:], den1[:])
                    sct1 = p_s.tile([1, RC], F32, tag="sct1")
                    nc.vector.tensor_mul(sct1[:], rec1[:], gcoef[:, rsl])
                    sct64 = p_s.tile([64, RC], F32, tag="sct64")
                    nc.gpsimd.partition_broadcast(sct64[:], sct1[:])
                    nc.vector.tensor_mul(
                        a2[b][mc][hh * 64:hh * 64 + 64, :],
                        aps[hh][0:64, :], sct64[:])

            def emit_outproj_block(a2p, r_prev, rb):
                row0 = r_prev * RC + rb * 128
                for half in range(2):
                    osb = p_o.tile([128, DLLM // 2], F32, tag="osb", name="osb")
                    for nh in range(4):
                        ncc = half * 4 + nh
                        nsl = slice(ncc * 512, (ncc + 1) * 512)
                        ops = psO.tile([128, 512], F32, tag="ops")
                        chains = [("t", 0), ("t", 1), ("d", 0), ("d", 1)]
                        for kk, (bb, mcc) in enumerate(chains):
                            nc.tensor.matmul(
                                ops[:], a2p[bb][mcc][:, rb * 128:(rb + 1) * 128],
                                wo_t[:, kk, nsl],
                                start=(kk == 0), stop=(kk == 3 and not with_bo))
                        if with_bo:
                            nc.tensor.matmul(
                                ops[:], g2_sb[:, row0:row0 + 128], bo_sb[:, nsl],
                                start=False, stop=True)
                        nc.vector.tensor_copy(osb[:, nh * 512:(nh + 1) * 512], ops[:])
                    nc.sync.dma_start(
                        out.ap()[row0:row0 + 128,
                                 half * (DLLM // 2):(half + 1) * (DLLM // 2)], osb[:])

            # phase B work for the last rows-chunk, emitted piecewise between
            # r=0's attention units as dense PE filler
            p_x2 = cctx.enter_context(tc.tile_pool(name="p_x2", bufs=1))
            bstate = {}   # rB -> (xtB dict, lacc)

            def emit_b3_load(rB):
                rslB = slice(rB * RC, (rB + 1) * RC)
                xtB = {}
                for b in "td":
                    xtB[b] = p_x2.tile([128, KD, RC], BF16, tag=f"xB_{b}",
                                       name=f"xB_{b}")
                    nc.sync.dma_start(
                        xtB[b][:],
                        xT[b].ap().rearrange("(c p) n -> p c n", p=128)[:, :, rslB])
                lacc = p_s.tile([1, RC], F32, tag="lacc", name="lacc")
                bstate[rB] = (xtB, lacc)

            def emit_b3_piece(rB, step):
                rslB = slice(rB * RC, (rB + 1) * RC)
                xtB, lacc = bstate[rB]
                # steps 0..7: gate hidden m-chunk (+ its logit partial);
                # 8: sigmoid; 9..12: qT chains
                if step < KD:
                    mc = step
                    hps = psO.tile([128, RC], F32, tag="ops", name="hpsB")
                    for kc in range(2 * KD):
                        nc.tensor.matmul(
                            hps[:],
                            w1_t[:, kc, mc * 128:(mc + 1) * 128],
                            xtB["t" if kc < KD else "d"][:, kc % KD, :],
                            start=(kc == 0), stop=(kc == 2 * KD - 1))
                    htmp = p_s.tile([128, RC], BF16, tag="htmp", name="htmp")
                    nc.scalar.activation(
                        htmp[:], hps[:], AF.Relu, bias=gb1_sb[:, mc:mc + 1])
                    lps = psO.tile([128, RC], F32, tag="ops", name="lpsB")
                    nc.tensor.matmul(lps[0:1, :], w2_t[:, mc, :], htmp[:],
                                     start=True, stop=True)
                    if mc == 0:
                        nc.vector.tensor_copy(lacc[:], lps[0:1, :])
                    else:
                        nc.vector.tensor_add(lacc[:], lacc[:], lps[0:1, :])
                elif step == KD:
                    nc.scalar.activation(
                        gate_sb[:, rslB], lacc[:], AF.Sigmoid, bias=gb2_sb[:])
                    nc.vector.tensor_sub(
                        omg_sb[:, rslB], onesrow[:], gate_sb[:, rslB])
                    if with_bo:
                        nc.vector.tensor_copy(g2_sb[0:1, rslB], gate_sb[:, rslB])
                        nc.vector.tensor_copy(g2_sb[1:2, rslB], omg_sb[:, rslB])
                else:
                    bq, mcq = divmod(step - KD - 1, 2)
                    bb = "t" if bq == 0 else "d"
                    qps = psO.tile([128, RC], F32, tag="ops", name="qpsB")
                    for kc in range(KD):
                        nc.tensor.matmul(
                            qps[:],
                            wq_t[bb][:, kc, mcq * 128:(mcq + 1) * 128],
                            xtB[bb][:, kc, :],
                            start=(kc == 0), stop=(kc == KD - 1))
                    nc.scalar.activation(
                        qt_sb[bb][:, mcq, rslB], qps[:], AF.Identity,
                        bias=bq_sb[:, (0 if bb == "t" else 2) + mcq:
                                   (0 if bb == "t" else 2) + mcq + 1])

            # deferred phase-B work for the last two rows-chunks: 26 pieces
            b3_tasks = []
            for rB in (NRC - 1,):
                b3_tasks.append(lambda rB=rB: emit_b3_load(rB))
                for s in range(KD + 5):
                    b3_tasks.append(lambda rB=rB, s=s: emit_b3_piece(rB, s))

            def make_outproj_tasks(a2p, r_prev):
                """One task per PSUM group; osb assembled per (rb, half)."""
                state = {}

                def group(rb, ncc):
                    row0 = r_prev * RC + rb * 128
                    half, nh = divmod(ncc, 4)
                    if nh == 0:
                        state[(rb, half)] = p_o.tile([128, DLLM // 2], F32,
                                                     tag="osb", name="osb")
                    osb = state[(rb, half)]
                    nsl = slice(ncc * 512, (ncc + 1) * 512)
                    ops = psO.tile([128, 512], F32, tag="ops", name="ops")
                    chains = [("t", 0), ("t", 1), ("d", 0), ("d", 1)]
                    for kk, (bb, mcc) in enumerate(chains):
                        nc.tensor.matmul(
                            ops[:], a2p[bb][mcc][:, rb * 128:(rb + 1) * 128],
                            wo_t[:, kk, nsl],
                            start=(kk == 0), stop=(kk == 3 and not with_bo))
                    if with_bo:
                        nc.tensor.matmul(
                            ops[:], g2_sb[:, row0:row0 + 128], bo_sb[:, nsl],
                            start=False, stop=True)
                    nc.vector.tensor_copy(osb[:, nh * 512:(nh + 1) * 512], ops[:])
                    if nh == 3:
                        nc.sync.dma_start(
                            out.ap()[row0:row0 + 128,
                                     half * (DLLM // 2):(half + 1) * (DLLM // 2)],
                            osb[:])

                return [(lambda rb=rb, ncc=ncc: group(rb, ncc))
                        for rb in range(4) for ncc in range(8)]

            pending = None   # (a2 dict, r) awaiting output projection
            for r in range(NRC):
                rsl = slice(r * RC, (r + 1) * RC)
                a2 = {b: [p_a.tile([128, RC], BF16, tag=f"a2_{b}{mc}",
                                   name=f"a2_{b}{mc}")
                          for mc in range(2)] for b in "td"}
                units = [(b, mc) for b in "td" for mc in range(2)]
                # filler: out-proj groups of r-1 (32 tasks), or the deferred
                # phase-B work for the last rows-chunk during r=0 (13 tasks)
                if pending is not None:
                    tasks = make_outproj_tasks(pending[0], pending[1])
                else:
                    tasks = b3_tasks
                ti = 0
                nslots = len(units) * NSC
                prev = None   # (aps, b, mc, p2dict)
                slot = 0
                for u, (b, mc) in enumerate(units):
                    aps = [psPV.tile([65, RC], F32, tag=f"aps{mc}{hh}",
                                     name=f"aps{mc}{hh}") for hh in range(2)]
                    p2buf = {}
                    for si in range(NSC):
                        p2buf[si] = emit_qk_exp(b, mc, si, rsl)
                        if prev is not None:
                            paps, pb, pmc, pp2 = prev
                            emit_pv(paps, pb, pmc, si, pp2[si])
                        # paced filler: one task per due slot
                        slot += 1
                        due = (slot * len(tasks)) // nslots
                        while ti < due:
                            tasks[ti]()
                            ti += 1
                    if prev is not None:
                        emit_norm(prev[0], prev[1], prev[2], a2, rsl)
                    prev = (aps, b, mc, p2buf)
                # drain last unit of this rows-chunk
                paps, pb, pmc, pp2 = prev
                for si in range(NSC):
                    emit_pv(paps, pb, pmc, si, pp2[si])
                emit_norm(paps, pb, pmc, a2, rsl)
                while ti < len(tasks):
                    tasks[ti]()
                    ti += 1
                pending = (a2, r)
            for task in make_outproj_tasks(pending[0], pending[1]):
                task()
        pre1ctx.close()

    nc.compile()
    return nc


def _prep_inputs(inputs):
    """Host-side shard + transpose. Returns in_maps for 8 cores."""
    f32 = np.float32
    t = {k: np.asarray(v) for k, v in inputs.items()}
    x_full = {"t": t["trend_emb"].reshape(B * L, D).astype(f32),
              "d": t["detail_emb"].reshape(B * L, D).astype(f32)}
    pT_full = {"t": np.ascontiguousarray(t["trend_proto"].astype(f32).T).astype(bf16),
               "d": np.ascontiguousarray(t["detail_proto"].astype(f32).T).astype(bf16)}
    W = {("q", "t"): t["t_Wq"], ("q", "d"): t["d_Wq"],
         ("k", "t"): t["t_Wk"], ("k", "d"): t["d_Wk"],
         ("v", "t"): t["t_Wv"], ("v", "d"): t["d_Wv"],
         ("o", "t"): t["t_Wo"], ("o", "d"): t["d_Wo"]}
    bias = {("q", "t"): t["t_bq"], ("q", "d"): t["d_bq"],
            ("k", "t"): t["t_bk"], ("k", "d"): t["d_bk"],
            ("v", "t"): t["t_bv"], ("v", "d"): t["d_bv"],
            ("o", "t"): t["t_bo"], ("o", "d"): t["d_bo"]}

    with_bo = bool(np.any(bias[("o", "t")]) or np.any(bias[("o", "d")]))
    in_maps = []
    for core in range(8):
        rg, hg = divmod(core, HG)
        rows = slice(rg * R, (rg + 1) * R)
        hsl = slice(hg * HEC, (hg + 1) * HEC)
        m = {}
        for b in "td":
            m[f"xT_{b}"] = np.ascontiguousarray(x_full[b][rows].T).astype(bf16)
            m[f"pT_{b}"] = pT_full[b]
            m[f"wq_{b}"] = np.ascontiguousarray(W[("q", b)][:, hsl]).astype(bf16)
            m[f"wk_{b}"] = np.ascontiguousarray(W[("k", b)][:, hsl]).astype(bf16)
            m[f"wv_{b}"] = np.ascontiguousarray(W[("v", b)][:, hsl]).astype(bf16)
        m["wo"] = np.vstack([W[("o", "t")][hsl, :], W[("o", "d")][hsl, :]]).astype(bf16)
        m["w1"] = t["g_W1"].astype(bf16)
        m["w2"] = t["g_W2"].astype(bf16)
        m["bq2"] = np.stack([bias[("q", "t")][hsl][0:128], bias[("q", "t")][hsl][128:256],
                             bias[("q", "d")][hsl][0:128], bias[("q", "d")][hsl][128:256]],
                            axis=1).astype(f32)
        m["bk2"] = np.stack([bias[("k", "t")][hsl][0:128], bias[("k", "t")][hsl][128:256],
                             bias[("k", "d")][hsl][0:128], bias[("k", "d")][hsl][128:256]],
                            axis=1).astype(f32)
        m["bv"] = np.concatenate([bias[("v", "t")][hsl],
                                  bias[("v", "d")][hsl]])[None, :].astype(bf16)
        m["gb1"] = np.ascontiguousarray(
            t["g_b1"].astype(f32).reshape(KD, 128).T)
        m["gb2"] = t["g_b2"].astype(f32).reshape(1, 1)
        m["ones"] = np.ones((1, 2048), f32)
        if with_bo:
            m["bo2"] = (np.stack([bias[("o", "t")], bias[("o", "d")]]) / HG).astype(bf16)
        in_maps.append(m)
    return in_maps, with_bo


def kernel(**inputs):
    global LAST_RESULTS
    import os
    from concourse.bass_utils import run_bass_kernel_spmd

    in_maps, with_bo = _prep_inputs(inputs)
    if with_bo not in _CACHE:
        _CACHE[with_bo] = _build(with_bo)
    nc = _CACHE[with_bo]

    trace = bool(os.environ.get("KERNEL_TRACE"))
    res = run_bass_kernel_spmd(
        nc, in_maps, list(range(8)),
        trace=trace, trace_cores=list(range(8)) if trace else None)
    LAST_RESULTS = res

    out = np.empty((RG, R, DLLM), np.float32)
    for rg in range(RG):
        acc = res.results[rg * HG]["out"].astype(np.float32)
        for hg in range(1, HG):
            acc = acc + res.results[rg * HG + hg]["out"]
        out[rg] = acc
    return out.reshape(B, L, DLLM)


# revision 39
# speedup vs baseline: 1.0024x; 1.0024x over previous
"""DualReprogrammingLayer Trainium2 kernel.

Sharding: 2 row-groups (B*L split in halves) x 4 head-groups (4 heads each).
Each core computes, for its 2048 rows and 4 heads (per block in {trend, detail}):
  KT = (Wk.T @ protoT)           (heads-slice, S)        [K-proj, f32r in, bf16 out]
  V  = (protoT.T @ Wv) + bv      (S, heads-slice)        [V-proj, f32r]
  qT = (Wq.T @ xT) + bq          (heads-slice, rows)     [bf16]
  scoresT = KT_h @ qT_h          (S, rows) per head      [bf16, 2-head row-packed]
  P  = exp(scoresT / 8)                                  [ACT, f32r out]
  A_ext = [V_h | ones].T @ P     (64+64, rows)           [f32r; rows 64:128 = denom]
  gate = sigmoid(relu(cat @ W1) @ W2)  (on-device, bf16)
  A_scaled = A * (gate_coef / denom)                     [bf16]
  out_partial = [A_t; A_d].T-stack @ [Wo_t; Wo_d]        [bf16]
Host sums the 4 head-group partials per row-group.
"""
import sys
sys.path.insert(0, '/opt/trn_rl_repo')
from contextlib import ExitStack

import numpy as np
import ml_dtypes

import concourse.bass as bass
import concourse.tile as tile
from concourse import bacc, mybir

F32 = mybir.dt.float32
F32R = mybir.dt.float32r
BF16 = mybir.dt.bfloat16
AF = mybir.ActivationFunctionType
bf16 = ml_dtypes.bfloat16

B, L, D, S, DLLM, H, E = 4, 1024, 1024, 1000, 4096, 16, 64
RG, HG = 2, 4                 # row-groups x head-groups = 8 cores
R = (B * L) // RG             # 2048 rows per core
NH = H // HG                  # 4 heads per core
HEC = NH * E                  # 256
SCH, NSC = 125, 8             # S = 8 chunks of 125
RC, NRC = 512, 4              # rows = 4 chunks of 512
KD = D // 128                 # 8 k-chunks for d_model
KL = DLLM // 128              # 32 k-chunks for d_llm

_CACHE = {}
LAST_RESULTS = None           # set by kernel(): BassKernelResults


def _build(with_bo):
    nc = bacc.Bacc("TRN2", target_bir_lowering=False, debug=False)

    def din(name, shape, dt):
        return nc.dram_tensor(name, list(shape), dt, kind="ExternalInput")

    xT = {b: din(f"xT_{b}", (D, R), BF16) for b in "td"}
    pT = {b: din(f"pT_{b}", (DLLM, S), BF16) for b in "td"}
    wq = {b: din(f"wq_{b}", (D, HEC), BF16) for b in "td"}
    wk = {b: din(f"wk_{b}", (DLLM, HEC), BF16) for b in "td"}
    wv = {b: din(f"wv_{b}", (DLLM, HEC), BF16) for b in "td"}
    wo = din("wo", (2 * HEC, DLLM), BF16)            # [t rows | d rows]
    w1 = din("w1", (2 * D, D), BF16)
    w2 = din("w2", (D, 1), BF16)
    bq2 = din("bq2", (128, 4), F32)                  # cols: t-mc0, t-mc1, d-mc0, d-mc1
    bk2 = din("bk2", (128, 4), F32)
    bvv = din("bv", (1, 2 * HEC), BF16)               # [t 256 | d 256]
    gb1 = din("gb1", (128, KD), F32)
    gb2 = din("gb2", (1, 1), F32)
    ones_d = din("ones", (1, 2048), F32)
    bo2 = din("bo2", (2, DLLM), BF16) if with_bo else None
    out = nc.dram_tensor("out", [R, DLLM], F32, kind="ExternalOutput")

    with tile.TileContext(nc) as tc, ExitStack() as ctx:
        # ---- persistent pools (live across phases) ----
        pers = ctx.enter_context(tc.tile_pool(name="pers", bufs=1))
        kt_sb = {}    # block -> tile (128, 2, S) bf16 : HE chunk mc at [:, mc, :]
        vx_sb = {}    # block -> tile (125, NSC, NH, 128) f32r : [V_h | ones]
        qt_sb = {}    # block -> tile (128, 2, R) bf16
        for b in "td":
            kt_sb[b] = pers.tile([128, 2, S], BF16, tag=f"kt_{b}", name=f"kt_{b}")
            vx_sb[b] = pers.tile([SCH, NSC, NH, 65], BF16, tag=f"vx_{b}", name=f"vx_{b}")
            qt_sb[b] = pers.tile([128, 2, R], BF16, tag=f"qt_{b}", name=f"qt_{b}")
        gate_sb = pers.tile([1, R], F32, tag="gate")     # sigmoid output
        omg_sb = pers.tile([1, R], F32, tag="omg")       # 1 - gate
        ones125 = pers.tile([1, SCH], BF16, tag="ones125")
        nc.vector.memset(ones125[:], 1.0)
        onesrow = pers.tile([1, RC], F32, tag="onesrow")
        nc.vector.memset(onesrow[:], 1.0)
        bq_sb = pers.tile([128, 4], F32, tag="bq")
        nc.gpsimd.dma_start(bq_sb[:], bq2.ap())
        bk_sb = pers.tile([128, 4], F32, tag="bk")
        nc.gpsimd.dma_start(bk_sb[:], bk2.ap())
        bv_sb = pers.tile([1, 2 * HEC], BF16, tag="bv")
        nc.gpsimd.dma_start(bv_sb[:], bvv.ap())
        gb1_sb = pers.tile([128, KD], F32, tag="gb1")
        nc.gpsimd.dma_start(gb1_sb[:], gb1.ap())
        gb2_sb = pers.tile([1, 1], F32, tag="gb2")
        nc.gpsimd.dma_start(gb2_sb[:], gb2.ap())
        if with_bo:
            bo_sb = pers.tile([2, DLLM], BF16, tag="bo")
            nc.gpsimd.dma_start(bo_sb[:], bo2.ap())
            g2_sb = pers.tile([2, R], BF16, tag="g2")

        # ---- prefetch pools: weights for later phases, loaded during phase A.
        # p_pre2 (wo) lives through phase C; p_pre1 (W1/wq) releases after B.
        p_pre2 = ctx.enter_context(tc.tile_pool(name="p_pre2", bufs=1))
        wo_t = p_pre2.tile([128, 4, DLLM], BF16, tag="wo")
        pre1ctx = ExitStack()
        p_pre1 = pre1ctx.enter_context(tc.tile_pool(name="p_pre1", bufs=1))
        w1_t = p_pre1.tile([128, 2 * KD, D], BF16, tag="w1")
        w2_t = p_pre1.tile([128, KD, 1], BF16, tag="w2")
        wq_t = {}
        for b in "td":
            wq_t[b] = p_pre1.tile([128, KD, HEC], BF16, tag=f"wq_{b}", name=f"wq_{b}")

        def emit_prefetch():
            nc.scalar.dma_start(w1_t[:], w1.ap().rearrange("(c p) m -> p c m", p=128))
            nc.scalar.dma_start(w2_t[:], w2.ap().rearrange("(c p) m -> p c m", p=128))
            for b in "td":
                nc.scalar.dma_start(
                    wq_t[b][:], wq[b].ap().rearrange("(c p) m -> p c m", p=128))
            nc.scalar.dma_start(wo_t[:], wo.ap().rearrange("(c p) n -> p c n", p=128))

        # ---- phase A: K/V projections (proto and weights streamed per k-chunk) ----
        with ExitStack() as actx:
            p_pt = actx.enter_context(tc.tile_pool(name="p_pt", bufs=8))
            p_wc = actx.enter_context(tc.tile_pool(name="p_wc", bufs=1))
            psA = actx.enter_context(tc.tile_pool(name="psA", bufs=1, space="PSUM"))
            # PE warm-up: ~8us of dummy matmuls on memset tiles so the HAM
            # clock-gate reaches 8/8 before the first real matmul, overlapping
            # the initial DMA ramp. Results are discarded.
            wu_l = p_wc.tile([64, 128], BF16, tag="wu_l")
            nc.vector.memset(wu_l[:], 0.0)
            wu_r = p_wc.tile([64, 512], BF16, tag="wu_r")
            nc.vector.memset(wu_r[:], 0.0)

            for b in "td":
                vps = [psA.tile([SCH, 2, HEC], F32, tag=f"vps{i}", name=f"vps{i}")
                       for i in range(4)]
                kps = [psA.tile([128, 512], F32, tag=f"kps{i}", name=f"kps{i}")
                       for i in range(4)]
                if b == "t":
                    # discarded warm-up group in kps[0]'s bank, closed before
                    # the real accumulation group opens
                    for wi in range(36):
                        nc.tensor.matmul(kps[0][:], wu_l[:], wu_r[:],
                                         start=(wi == 0), stop=(wi == 35))
                wk_t = p_wc.tile([128, KL, HEC], BF16, tag="wk_t")
                wv_t = p_wc.tile([128, KL, HEC], BF16, tag="wv_t")
                wk_r4 = wk[b].ap().rearrange("(c p) m -> p c m", p=128)
                wv_r4 = wv[b].ap().rearrange("(c p) m -> p c m", p=128)
                pt_r = pT[b].ap().rearrange("(c p) s -> c p s", c=KL)
                for kc in range(KL):
                    if kc % (KL // 2) == 0:
                        hq = slice(kc, kc + KL // 2)
                        nc.scalar.dma_start(wv_t[:, hq, :], wv_r4[:, hq, :])
                        nc.scalar.dma_start(wk_t[:, hq, :], wk_r4[:, hq, :])
                    pt_t = p_pt.tile([128, S], BF16, tag="pt")
                    nc.sync.dma_start(pt_t[:], pt_r[kc])
                    wkc = wk_t[:, kc, :]
                    wvc = wv_t[:, kc, :]
                    for si in range(NSC):
                        # one accumulation group per PSUM bank: only the first
                        # half issues start=True (bank-wide clear covers both)
                        nc.tensor.matmul(
                            vps[si // 2][:, si % 2, :],
                            pt_t[:, si * SCH:(si + 1) * SCH],
                            wvc,
                            start=(kc == 0 and si % 2 == 0), stop=False)
                    for mc in range(2):
                        for ncc in range(2):
                            nc.tensor.matmul(
                                kps[mc * 2 + ncc][:, 0:500],
                                wk_t[:, kc, mc * 128:(mc + 1) * 128],
                                pt_t[:, ncc * 500:(ncc + 1) * 500],
                                start=(kc == 0), stop=(kc == KL - 1))
                boff = 0 if b == "t" else HEC
                for si in range(NSC):
                    nc.tensor.matmul(
                        vps[si // 2][:, si % 2, :],
                        ones125[:],
                        bv_sb[:, boff:boff + HEC],
                        start=False, stop=(si % 2 == 1))
                for si in range(NSC):
                    # copy V psum (125, 256) -> [:, si, :, 0:64] viewed as (125, 4, 64)
                    nc.vector.tensor_copy(
                        vx_sb[b][:, si, :, 0:64],
                        vps[si // 2][:, si % 2, :].rearrange("p (h e) -> p h e", h=NH))
                nc.vector.memset(vx_sb[b][:, :, :, 64:65], 1.0)
                for mc in range(2):
                    for ncc in range(2):
                        nc.scalar.activation(
                            kt_sb[b][:, mc, ncc * 500:(ncc + 1) * 500],
                            kps[mc * 2 + ncc][:, 0:500],
                            AF.Identity,
                            bias=bk_sb[:, (0 if b == "t" else 2) + mc:
                                       (0 if b == "t" else 2) + mc + 1])
                if b == "t":
                    emit_prefetch()

        # ---- phase B: gate + Q projections (per rows-chunk) ----
        with ExitStack() as bctx:
            p_x = bctx.enter_context(tc.tile_pool(name="p_x", bufs=3))
            p_h = bctx.enter_context(tc.tile_pool(name="p_h", bufs=2))
            psB = bctx.enter_context(tc.tile_pool(name="psB", bufs=2, space="PSUM"))
            for r in range(NRC - 1):
                rsl = slice(r * RC, (r + 1) * RC)
                xt = {}
                for b in "td":
                    xt[b] = p_x.tile([128, KD, RC], BF16, tag=f"x_{b}", name=f"x_{b}")
                    nc.sync.dma_start(
                        xt[b][:],
                        xT[b].ap().rearrange("(c p) n -> p c n", p=128)[:, :, rsl])
                # gate hidden: 8 m-chunks, contraction over 16 chunks (t then d)
                ht = p_h.tile([128, KD, RC], BF16, tag="ht")
                for mc in range(KD):
                    hps = psB.tile([128, RC], F32, tag="hps")
                    for kc in range(2 * KD):
                        nc.tensor.matmul(
                            hps[:],
                            w1_t[:, kc, mc * 128:(mc + 1) * 128],
                            xt["t" if kc < KD else "d"][:, kc % KD, :],
                            start=(kc == 0), stop=(kc == 2 * KD - 1))
                    nc.scalar.activation(
                        ht[:, mc, :], hps[:], AF.Relu,
                        bias=gb1_sb[:, mc:mc + 1])
                lps = psB.tile([1, RC], F32, tag="lps")
                for mc in range(KD):
                    nc.tensor.matmul(
                        lps[:], w2_t[:, mc, :], ht[:, mc, :],
                        start=(mc == 0), stop=(mc == KD - 1))
                nc.scalar.activation(
                    gate_sb[:, rsl], lps[:], AF.Sigmoid, bias=gb2_sb[:])
                nc.vector.tensor_sub(omg_sb[:, rsl], onesrow[:], gate_sb[:, rsl])
                if with_bo:
                    nc.vector.tensor_copy(g2_sb[0:1, rsl], gate_sb[:, rsl])
                    nc.vector.tensor_copy(g2_sb[1:2, rsl], omg_sb[:, rsl])
                # Q projections
                for b in "td":
                    for mc in range(2):
                        qps = psB.tile([128, RC], F32, tag="qps")
                        for kc in range(KD):
                            nc.tensor.matmul(
                                qps[:],
                                wq_t[b][:, kc, mc * 128:(mc + 1) * 128],
                                xt[b][:, kc, :],
                                start=(kc == 0), stop=(kc == KD - 1))
                        nc.scalar.activation(
                            qt_sb[b][:, mc, rsl], qps[:], AF.Identity,
                            bias=bq_sb[:, (0 if b == "t" else 2) + mc:
                                       (0 if b == "t" else 2) + mc + 1])
        # ---- phase C: attention + output projection ----
        # Software pipeline: QK+exp of unit u overlaps PV of unit u-1 at
        # s-chunk granularity; output-projection rows-blocks of the previous
        # rows-chunk interleave between units to keep the PE stream dense.
        with ExitStack() as cctx:
            p_p = cctx.enter_context(tc.tile_pool(name="p_p", bufs=4))
            p_a = cctx.enter_context(tc.tile_pool(name="p_a", bufs=2))
            p_s = cctx.enter_context(tc.tile_pool(name="p_s", bufs=2))
            p_o = cctx.enter_context(tc.tile_pool(name="p_o", bufs=2))
            psS = cctx.enter_context(tc.tile_pool(name="psS", bufs=1, space="PSUM"))
            psPV = cctx.enter_context(tc.tile_pool(name="psPV", bufs=1, space="PSUM"))
            psO = cctx.enter_context(tc.tile_pool(name="psO", bufs=2, space="PSUM"))

            def emit_qk_exp(b, mc, si, rsl):
                sps2 = psS.tile([SCH, 2, RC], F32, tag="sps", name="sps")
                for hh in range(2):  # row-packed pair, adjacent emission
                    po = hh * 64
                    nc.tensor.matmul(
                        sps2[:, hh, :],
                        kt_sb[b][po:po + 64, mc, si * SCH:(si + 1) * SCH],
                        qt_sb[b][po:po + 64, mc, rsl],
                        start=True, stop=True,
                        tile_position=(po, 0))
                p2 = p_p.tile([SCH, 2, RC], BF16, tag=f"p{si % 2}",
                              name=f"p{si % 2}")
                nc.scalar.activation(p2[:], sps2[:], AF.Exp, scale=0.125)
                return p2

            def emit_pv(aps, b, mc, si, p2):
                for hh in range(2):
                    h = mc * 2 + hh
                    nc.tensor.matmul(
                        aps[hh][:], vx_sb[b][:, si, h, :], p2[:, hh, :],
                        start=(si == 0), stop=(si == NSC - 1))

            def emit_norm(aps, b, mc, a2, rsl):
                gcoef = gate_sb if b == "t" else omg_sb
                for hh in range(2):
                    den1 = p_s.tile([1, RC], F32, tag="den1")
                    nc.vector.tensor_copy(den1[:], aps[hh][64:65, :])
                    rec1 = p_s.tile([1, RC], F32, tag="rec1")
                    nc.vector.reciprocal_approx_fast(rec1[:], den1[:])
                    sct1 = p_s.tile([1, RC], F32, tag="sct1")
                    nc.vector.tensor_mul(sct1[:], rec1[:], gcoef[:, rsl])
                    sct64 = p_s.tile([64, RC], F32, tag="sct64")
                    nc.gpsimd.partition_broadcast(sct64[:], sct1[:])
                    nc.vector.tensor_mul(
                        a2[b][mc][hh * 64:hh * 64 + 64, :],
                        aps[hh][0:64, :], sct64[:])

            def emit_outproj_block(a2p, r_prev, rb):
                row0 = r_prev * RC + rb * 128
                for half in range(2):
                    osb = p_o.tile([128, DLLM // 2], F32, tag="osb", name="osb")
                    for nh in range(4):
                        ncc = half * 4 + nh
                        nsl = slice(ncc * 512, (ncc + 1) * 512)
                        ops = psO.tile([128, 512], F32, tag="ops")
                        chains = [("t", 0), ("t", 1), ("d", 0), ("d", 1)]
                        for kk, (bb, mcc) in enumerate(chains):
                            nc.tensor.matmul(
                                ops[:], a2p[bb][mcc][:, rb * 128:(rb + 1) * 128],
                                wo_t[:, kk, nsl],
                                start=(kk == 0), stop=(kk == 3 and not with_bo))
                        if with_bo:
                            nc.tensor.matmul(
                                ops[:], g2_sb[:, row0:row0 + 128], bo_sb[:, nsl],
                                start=False, stop=True)
                        nc.vector.tensor_copy(osb[:, nh * 512:(nh + 1) * 512], ops[:])
                    nc.sync.dma_start(
                        out.ap()[row0:row0 + 128,
                                 half * (DLLM // 2):(half + 1) * (DLLM // 2)], osb[:])

            # phase B work for the last rows-chunk, emitted piecewise between
            # r=0's attention units as dense PE filler
            p_x2 = cctx.enter_context(tc.tile_pool(name="p_x2", bufs=1))
            bstate = {}   # rB -> (xtB dict, lacc)

            def emit_b3_load(rB):
                rslB = slice(rB * RC, (rB + 1) * RC)
                xtB = {}
                for b in "td":
                    xtB[b] = p_x2.tile([128, KD, RC], BF16, tag=f"xB_{b}",
                                       name=f"xB_{b}")
                    nc.sync.dma_start(
                        xtB[b][:],
                        xT[b].ap().rearrange("(c p) n -> p c n", p=128)[:, :, rslB])
                lacc = p_s.tile([1, RC], F32, tag="lacc", name="lacc")
                bstate[rB] = (xtB, lacc)

            def emit_b3_piece(rB, step):
                rslB = slice(rB * RC, (rB + 1) * RC)
                xtB, lacc = bstate[rB]
                # steps 0..7: gate hidden m-chunk (+ its logit partial);
                # 8: sigmoid; 9..12: qT chains
                if step < KD:
                    mc = step
                    hps = psO.tile([128, RC], F32, tag="ops", name="hpsB")
                    for kc in range(2 * KD):
                        nc.tensor.matmul(
                            hps[:],
                            w1_t[:, kc, mc * 128:(mc + 1) * 128],
                            xtB["t" if kc < KD else "d"][:, kc % KD, :],
                            start=(kc == 0), stop=(kc == 2 * KD - 1))
                    htmp = p_s.tile([128, RC], BF16, tag="htmp", name="htmp")
                    nc.scalar.activation(
                        htmp[:], hps[:], AF.Relu, bias=gb1_sb[:, mc:mc + 1])
                    lps = psO.tile([128, RC], F32, tag="ops", name="lpsB")
                    nc.tensor.matmul(lps[0:1, :], w2_t[:, mc, :], htmp[:],
                                     start=True, stop=True)
                    if mc == 0:
                        nc.vector.tensor_copy(lacc[:], lps[0:1, :])
                    else:
                        nc.vector.tensor_add(lacc[:], lacc[:], lps[0:1, :])
                elif step == KD:
                    nc.scalar.activation(
                        gate_sb[:, rslB], lacc[:], AF.Sigmoid, bias=gb2_sb[:])
                    nc.vector.tensor_sub(
                        omg_sb[:, rslB], onesrow[:], gate_sb[:, rslB])
                    if with_bo:
                        nc.vector.tensor_copy(g2_sb[0:1, rslB], gate_sb[:, rslB])
                        nc.vector.tensor_copy(g2_sb[1:2, rslB], omg_sb[:, rslB])
                else:
                    bq, mcq = divmod(step - KD - 1, 2)
                    bb = "t" if bq == 0 else "d"
                    qps = psO.tile([128, RC], F32, tag="ops", name="qpsB")
                    for kc in range(KD):
                        nc.tensor.matmul(
                            qps[:],
                            wq_t[bb][:, kc, mcq * 128:(mcq + 1) * 128],
                            xtB[bb][:, kc, :],
                            start=(kc == 0), stop=(kc == KD - 1))
                    nc.scalar.activation(
                        qt_sb[bb][:, mcq, rslB], qps[:], AF.Identity,
                        bias=bq_sb[:, (0 if bb == "t" else 2) + mcq:
                                   (0 if bb == "t" else 2) + mcq + 1])

            # deferred phase-B work for the last two rows-chunks: 26 pieces
            b3_tasks = []
            for rB in (NRC - 1,):
                b3_tasks.append(lambda rB=rB: emit_b3_load(rB))
                for s in range(KD + 5):
                    b3_tasks.append(lambda rB=rB, s=s: emit_b3_piece(rB, s))

            def make_outproj_tasks(a2p, r_prev):
                """One task per PSUM group; osb assembled per (rb, half)."""
                state = {}

                def group(rb, ncc):
                    row0 = r_prev * RC + rb * 128
                    half, nh = divmod(ncc, 4)
                    if nh == 0:
                        state[(rb, half)] = p_o.tile([128, DLLM // 2], F32,
                                                     tag="osb", name="osb")
                    osb = state[(rb, half)]
                    nsl = slice(ncc * 512, (ncc + 1) * 512)
                    ops = psO.tile([128, 512], F32, tag="ops", name="ops")
                    chains = [("t", 0), ("t", 1), ("d", 0), ("d", 1)]
                    for kk, (bb, mcc) in enumerate(chains):
                        nc.tensor.matmul(
                            ops[:], a2p[bb][mcc][:, rb * 128:(rb + 1) * 128],
                            wo_t[:, kk, nsl],
                            start=(kk == 0), stop=(kk == 3 and not with_bo))
                    if with_bo:
                        nc.tensor.matmul(
                            ops[:], g2_sb[:, row0:row0 + 128], bo_sb[:, nsl],
                            start=False, stop=True)
                    nc.vector.tensor_copy(osb[:, nh * 512:(nh + 1) * 512], ops[:])
                    if nh == 3:
                        nc.sync.dma_start(
                            out.ap()[row0:row0 + 128,
                                     half * (DLLM // 2):(half + 1) * (DLLM // 2)],
                            osb[:])

                return [(lambda rb=rb, ncc=ncc: group(rb, ncc))
                        for rb in range(4) for ncc in range(8)]

            pending = None   # (a2 dict, r) awaiting output projection
            for r in range(NRC):
                rsl = slice(r * RC, (r + 1) * RC)
                a2 = {b: [p_a.tile([128, RC], BF16, tag=f"a2_{b}{mc}",
                                   name=f"a2_{b}{mc}")
                          for mc in range(2)] for b in "td"}
                units = [(b, mc) for b in "td" for mc in range(2)]
                # filler: out-proj groups of r-1 (32 tasks), or the deferred
                # phase-B work for the last rows-chunk during r=0 (13 tasks)
                if pending is not None:
                    tasks = make_outproj_tasks(pending[0], pending[1])
                else:
                    tasks = b3_tasks
                ti = 0
                nslots = len(units) * NSC
                prev = None   # (aps, b, mc, p2dict)
                slot = 0
                for u, (b, mc) in enumerate(units):
                    aps = [psPV.tile([65, RC], F32, tag=f"aps{mc}{hh}",
                                     name=f"aps{mc}{hh}") for hh in range(2)]
                    p2buf = {}
                    for si in range(NSC):
                        p2buf[si] = emit_qk_exp(b, mc, si, rsl)
                        if prev is not None:
                            paps, pb, pmc, pp2 = prev
                            emit_pv(paps, pb, pmc, si, pp2[si])
                        # paced filler: one task per due slot
                        slot += 1
                        due = (slot * len(tasks)) // nslots
                        while ti < due:
                            tasks[ti]()
                            ti += 1
                    if prev is not None:
                        emit_norm(prev[0], prev[1], prev[2], a2, rsl)
                    prev = (aps, b, mc, p2buf)
                # drain last unit of this rows-chunk
                paps, pb, pmc, pp2 = prev
                for si in range(NSC):
                    emit_pv(paps, pb, pmc, si, pp2[si])
                emit_norm(paps, pb, pmc, a2, rsl)
                while ti < len(tasks):
                    tasks[ti]()
                    ti += 1
                pending = (a2, r)
            for task in make_outproj_tasks(pending[0], pending[1]):
                task()
        pre1ctx.close()

    nc.compile()
    return nc


def _prep_inputs(inputs):
    """Host-side shard + transpose. Returns in_maps for 8 cores."""
    f32 = np.float32
    t = {k: np.asarray(v) for k, v in inputs.items()}
    x_full = {"t": t["trend_emb"].reshape(B * L, D).astype(f32),
              "d": t["detail_emb"].reshape(B * L, D).astype(f32)}
    pT_full = {"t": np.ascontiguousarray(t["trend_proto"].astype(f32).T).astype(bf16),
               "d": np.ascontiguousarray(t["detail_proto"].astype(f32).T).astype(bf16)}
    W = {("q", "t"): t["t_Wq"], ("q", "d"): t["d_Wq"],
         ("k", "t"): t["t_Wk"], ("k", "d"): t["d_Wk"],
         ("v", "t"): t["t_Wv"], ("v", "d"): t["d_Wv"],
         ("o", "t"): t["t_Wo"], ("o", "d"): t["d_Wo"]}
    bias = {("q", "t"): t["t_bq"], ("q", "d"): t["d_bq"],
            ("k", "t"): t["t_bk"], ("k", "d"): t["d_bk"],
            ("v", "t"): t["t_bv"], ("v", "d"): t["d_bv"],
            ("o", "t"): t["t_bo"], ("o", "d"): t["d_bo"]}

    with_bo = bool(np.any(bias[("o", "t")]) or np.any(bias[("o", "d")]))
    in_maps = []
    for core in range(8):
        rg, hg = divmod(core, HG)
        rows = slice(rg * R, (rg + 1) * R)
        hsl = slice(hg * HEC, (hg + 1) * HEC)
        m = {}
        for b in "td":
            m[f"xT_{b}"] = np.ascontiguousarray(x_full[b][rows].T).astype(bf16)
            m[f"pT_{b}"] = pT_full[b]
            m[f"wq_{b}"] = np.ascontiguousarray(W[("q", b)][:, hsl]).astype(bf16)
            m[f"wk_{b}"] = np.ascontiguousarray(W[("k", b)][:, hsl]).astype(bf16)
            m[f"wv_{b}"] = np.ascontiguousarray(W[("v", b)][:, hsl]).astype(bf16)
        m["wo"] = np.vstack([W[("o", "t")][hsl, :], W[("o", "d")][hsl, :]]).astype(bf16)
        m["w1"] = t["g_W1"].astype(bf16)
        m["w2"] = t["g_W2"].astype(bf16)
        m["bq2"] = np.stack([bias[("q", "t")][hsl][0:128], bias[("q", "t")][hsl][128:256],
                             bias[("q", "d")][hsl][0:128], bias[("q", "d")][hsl][128:256]],
                            axis=1).astype(f32)
        m["bk2"] = np.stack([bias[("k", "t")][hsl][0:128], bias[("k", "t")][hsl][128:256],
                             bias[("k", "d")][hsl][0:128], bias[("k", "d")][hsl][128:256]],
                            axis=1).astype(f32)
        m["bv"] = np.concatenate([bias[("v", "t")][hsl],
                                  bias[("v", "d")][hsl]])[None, :].astype(bf16)
        m["gb1"] = np.ascontiguousarray(
            t["g_b1"].astype(f32).reshape(KD, 128).T)
        m["gb2"] = t["g_b2"].astype(f32).reshape(1, 1)
        m["ones"] = np.ones((1, 2048), f32)
        if with_bo:
            m["bo2"] = (np.stack([bias[("o", "t")], bias[("o", "d")]]) / HG).astype(bf16)
        in_maps.append(m)
    return in_maps, with_bo


def kernel(**inputs):
    global LAST_RESULTS
    import os
    from concourse.bass_utils import run_bass_kernel_spmd

    in_maps, with_bo = _prep_inputs(inputs)
    if with_bo not in _CACHE:
        _CACHE[with_bo] = _build(with_bo)
    nc = _CACHE[with_bo]

    trace = bool(os.environ.get("KERNEL_TRACE"))
    res = run_bass_kernel_spmd(
        nc, in_maps, list(range(8)),
        trace=trace, trace_cores=list(range(8)) if trace else None)
    LAST_RESULTS = res

    out = np.empty((RG, R, DLLM), np.float32)
    for rg in range(RG):
        acc = res.results[rg * HG]["out"].astype(np.float32)
        for hg in range(1, HG):
            acc = acc + res.results[rg * HG + hg]["out"]
        out[rg] = acc
    return out.reshape(B, L, DLLM)


# revision 40
# speedup vs baseline: 1.0071x; 1.0047x over previous
"""DualReprogrammingLayer Trainium2 kernel.

Sharding: 2 row-groups (B*L split in halves) x 4 head-groups (4 heads each).
Each core computes, for its 2048 rows and 4 heads (per block in {trend, detail}):
  KT = (Wk.T @ protoT)           (heads-slice, S)        [K-proj, f32r in, bf16 out]
  V  = (protoT.T @ Wv) + bv      (S, heads-slice)        [V-proj, f32r]
  qT = (Wq.T @ xT) + bq          (heads-slice, rows)     [bf16]
  scoresT = KT_h @ qT_h          (S, rows) per head      [bf16, 2-head row-packed]
  P  = exp(scoresT / 8)                                  [ACT, f32r out]
  A_ext = [V_h | ones].T @ P     (64+64, rows)           [f32r; rows 64:128 = denom]
  gate = sigmoid(relu(cat @ W1) @ W2)  (on-device, bf16)
  A_scaled = A * (gate_coef / denom)                     [bf16]
  out_partial = [A_t; A_d].T-stack @ [Wo_t; Wo_d]        [bf16]
Host sums the 4 head-group partials per row-group.
"""
import sys
sys.path.insert(0, '/opt/trn_rl_repo')
from contextlib import ExitStack

import numpy as np
import ml_dtypes

import concourse.bass as bass
import concourse.tile as tile
from concourse import bacc, mybir

F32 = mybir.dt.float32
F32R = mybir.dt.float32r
BF16 = mybir.dt.bfloat16
AF = mybir.ActivationFunctionType
bf16 = ml_dtypes.bfloat16

B, L, D, S, DLLM, H, E = 4, 1024, 1024, 1000, 4096, 16, 64
RG, HG = 2, 4                 # row-groups x head-groups = 8 cores
R = (B * L) // RG             # 2048 rows per core
NH = H // HG                  # 4 heads per core
HEC = NH * E                  # 256
SCH, NSC = 125, 8             # S = 8 chunks of 125
RC, NRC = 512, 4              # rows = 4 chunks of 512
KD = D // 128                 # 8 k-chunks for d_model
KL = DLLM // 128              # 32 k-chunks for d_llm

_CACHE = {}
LAST_RESULTS = None           # set by kernel(): BassKernelResults


def _build(with_bo):
    nc = bacc.Bacc("TRN2", target_bir_lowering=False, debug=False)

    def din(name, shape, dt):
        return nc.dram_tensor(name, list(shape), dt, kind="ExternalInput")

    xT = {b: din(f"xT_{b}", (D, R), BF16) for b in "td"}
    pT = {b: din(f"pT_{b}", (DLLM, S), BF16) for b in "td"}
    wq = {b: din(f"wq_{b}", (D, HEC), BF16) for b in "td"}
    wk = {b: din(f"wk_{b}", (DLLM, HEC), BF16) for b in "td"}
    wv = {b: din(f"wv_{b}", (DLLM, HEC), BF16) for b in "td"}
    wo = din("wo", (2 * HEC, DLLM), BF16)            # [t rows | d rows]
    w1 = din("w1", (2 * D, D), BF16)
    w2 = din("w2", (D, 1), BF16)
    bq2 = din("bq2", (128, 4), F32)                  # cols: t-mc0, t-mc1, d-mc0, d-mc1
    bk2 = din("bk2", (128, 4), F32)
    bvv = din("bv", (1, 2 * HEC), BF16)               # [t 256 | d 256]
    gb1 = din("gb1", (128, KD), F32)
    gb2 = din("gb2", (1, 1), F32)
    ones_d = din("ones", (1, 2048), F32)
    bo2 = din("bo2", (2, DLLM), BF16) if with_bo else None
    out = nc.dram_tensor("out", [R, DLLM], F32, kind="ExternalOutput")

    with tile.TileContext(nc) as tc, ExitStack() as ctx:
        # ---- persistent pools (live across phases) ----
        pers = ctx.enter_context(tc.tile_pool(name="pers", bufs=1))
        kt_sb = {}    # block -> tile (128, 2, S) bf16 : HE chunk mc at [:, mc, :]
        vx_sb = {}    # block -> tile (125, NSC, NH, 128) f32r : [V_h | ones]
        qt_sb = {}    # block -> tile (128, 2, R) bf16
        for b in "td":
            kt_sb[b] = pers.tile([128, 2, S], BF16, tag=f"kt_{b}", name=f"kt_{b}")
            vx_sb[b] = pers.tile([SCH, NSC, NH, 65], BF16, tag=f"vx_{b}", name=f"vx_{b}")
            qt_sb[b] = pers.tile([128, 2, R], BF16, tag=f"qt_{b}", name=f"qt_{b}")
        gate_sb = pers.tile([1, R], F32, tag="gate")     # sigmoid output
        omg_sb = pers.tile([1, R], F32, tag="omg")       # 1 - gate
        ones125 = pers.tile([1, SCH], BF16, tag="ones125")
        nc.vector.memset(ones125[:], 1.0)
        onesrow = pers.tile([1, RC], F32, tag="onesrow")
        nc.vector.memset(onesrow[:], 1.0)
        bq_sb = pers.tile([128, 4], F32, tag="bq")
        nc.gpsimd.dma_start(bq_sb[:], bq2.ap())
        bk_sb = pers.tile([128, 4], F32, tag="bk")
        nc.gpsimd.dma_start(bk_sb[:], bk2.ap())
        bv_sb = pers.tile([1, 2 * HEC], BF16, tag="bv")
        nc.gpsimd.dma_start(bv_sb[:], bvv.ap())
        gb1_sb = pers.tile([128, KD], F32, tag="gb1")
        nc.gpsimd.dma_start(gb1_sb[:], gb1.ap())
        gb2_sb = pers.tile([1, 1], F32, tag="gb2")
        nc.gpsimd.dma_start(gb2_sb[:], gb2.ap())
        if with_bo:
            bo_sb = pers.tile([2, DLLM], BF16, tag="bo")
            nc.gpsimd.dma_start(bo_sb[:], bo2.ap())
            g2_sb = pers.tile([2, R], BF16, tag="g2")

        # ---- prefetch pools: weights for later phases, loaded during phase A.
        # p_pre2 (wo) lives through phase C; p_pre1 (W1/wq) releases after B.
        p_pre2 = ctx.enter_context(tc.tile_pool(name="p_pre2", bufs=1))
        wo_t = p_pre2.tile([128, 4, DLLM], BF16, tag="wo")
        pre1ctx = ExitStack()
        p_pre1 = pre1ctx.enter_context(tc.tile_pool(name="p_pre1", bufs=1))
        w1_t = p_pre1.tile([128, 2 * KD, D], BF16, tag="w1")
        w2_t = p_pre1.tile([128, KD, 1], BF16, tag="w2")
        wq_t = {}
        for b in "td":
            wq_t[b] = p_pre1.tile([128, KD, HEC], BF16, tag=f"wq_{b}", name=f"wq_{b}")

        def emit_prefetch():
            nc.scalar.dma_start(w1_t[:], w1.ap().rearrange("(c p) m -> p c m", p=128))
            nc.scalar.dma_start(w2_t[:], w2.ap().rearrange("(c p) m -> p c m", p=128))
            for b in "td":
                nc.scalar.dma_start(
                    wq_t[b][:], wq[b].ap().rearrange("(c p) m -> p c m", p=128))
            nc.scalar.dma_start(wo_t[:], wo.ap().rearrange("(c p) n -> p c n", p=128))

        # ---- phase A: K/V projections (proto and weights streamed per k-chunk) ----
        with ExitStack() as actx:
            p_pt = actx.enter_context(tc.tile_pool(name="p_pt", bufs=6))
            p_wc = actx.enter_context(tc.tile_pool(name="p_wc", bufs=2))
            psA = actx.enter_context(tc.tile_pool(name="psA", bufs=1, space="PSUM"))
            # PE warm-up: ~8us of dummy matmuls on memset tiles so the HAM
            # clock-gate reaches 8/8 before the first real matmul, overlapping
            # the initial DMA ramp. Results are discarded.
            wu_l = p_wc.tile([64, 128], BF16, tag="wu_l")
            nc.vector.memset(wu_l[:], 0.0)
            wu_r = p_wc.tile([64, 512], BF16, tag="wu_r")
            nc.vector.memset(wu_r[:], 0.0)

            for b in "td":
                vps = [psA.tile([SCH, 2, HEC], F32, tag=f"vps{i}", name=f"vps{i}")
                       for i in range(4)]
                kps = [psA.tile([128, 512], F32, tag=f"kps{i}", name=f"kps{i}")
                       for i in range(4)]
                if b == "t":
                    # discarded warm-up group in kps[0]'s bank, closed before
                    # the real accumulation group opens
                    for wi in range(36):
                        nc.tensor.matmul(kps[0][:], wu_l[:], wu_r[:],
                                         start=(wi == 0), stop=(wi == 35))
                wk_t = p_wc.tile([128, KL, HEC], BF16, tag="wk_t")
                wv_t = p_wc.tile([128, KL, HEC], BF16, tag="wv_t")
                wk_r4 = wk[b].ap().rearrange("(c p) m -> p c m", p=128)
                wv_r4 = wv[b].ap().rearrange("(c p) m -> p c m", p=128)
                pt_r = pT[b].ap().rearrange("(c p) s -> c p s", c=KL)
                for kc in range(KL):
                    if kc % (KL // 2) == 0:
                        hq = slice(kc, kc + KL // 2)
                        nc.scalar.dma_start(wv_t[:, hq, :], wv_r4[:, hq, :])
                        nc.scalar.dma_start(wk_t[:, hq, :], wk_r4[:, hq, :])
                    pt_t = p_pt.tile([128, S], BF16, tag="pt")
                    nc.sync.dma_start(pt_t[:], pt_r[kc])
                    wkc = wk_t[:, kc, :]
                    wvc = wv_t[:, kc, :]
                    for si in range(NSC):
                        # one accumulation group per PSUM bank: only the first
                        # half issues start=True (bank-wide clear covers both)
                        nc.tensor.matmul(
                            vps[si // 2][:, si % 2, :],
                            pt_t[:, si * SCH:(si + 1) * SCH],
                            wvc,
                            start=(kc == 0 and si % 2 == 0), stop=False)
                    for mc in range(2):
                        for ncc in range(2):
                            nc.tensor.matmul(
                                kps[mc * 2 + ncc][:, 0:500],
                                wk_t[:, kc, mc * 128:(mc + 1) * 128],
                                pt_t[:, ncc * 500:(ncc + 1) * 500],
                                start=(kc == 0), stop=(kc == KL - 1))
                boff = 0 if b == "t" else HEC
                for si in range(NSC):
                    nc.tensor.matmul(
                        vps[si // 2][:, si % 2, :],
                        ones125[:],
                        bv_sb[:, boff:boff + HEC],
                        start=False, stop=(si % 2 == 1))
                for si in range(NSC):
                    # copy V psum (125, 256) -> [:, si, :, 0:64] viewed as (125, 4, 64)
                    nc.vector.tensor_copy(
                        vx_sb[b][:, si, :, 0:64],
                        vps[si // 2][:, si % 2, :].rearrange("p (h e) -> p h e", h=NH))
                nc.vector.memset(vx_sb[b][:, :, :, 64:65], 1.0)
                for mc in range(2):
                    for ncc in range(2):
                        nc.scalar.activation(
                            kt_sb[b][:, mc, ncc * 500:(ncc + 1) * 500],
                            kps[mc * 2 + ncc][:, 0:500],
                            AF.Identity,
                            bias=bk_sb[:, (0 if b == "t" else 2) + mc:
                                       (0 if b == "t" else 2) + mc + 1])
                if b == "t":
                    emit_prefetch()

        # ---- phase B: gate + Q projections (per rows-chunk) ----
        with ExitStack() as bctx:
            p_x = bctx.enter_context(tc.tile_pool(name="p_x", bufs=3))
            p_h = bctx.enter_context(tc.tile_pool(name="p_h", bufs=2))
            psB = bctx.enter_context(tc.tile_pool(name="psB", bufs=2, space="PSUM"))
            for r in range(NRC - 1):
                rsl = slice(r * RC, (r + 1) * RC)
                xt = {}
                for b in "td":
                    xt[b] = p_x.tile([128, KD, RC], BF16, tag=f"x_{b}", name=f"x_{b}")
                    nc.sync.dma_start(
                        xt[b][:],
                        xT[b].ap().rearrange("(c p) n -> p c n", p=128)[:, :, rsl])
                # gate hidden: 8 m-chunks, contraction over 16 chunks (t then d)
                ht = p_h.tile([128, KD, RC], BF16, tag="ht")
                for mc in range(KD):
                    hps = psB.tile([128, RC], F32, tag="hps")
                    for kc in range(2 * KD):
                        nc.tensor.matmul(
                            hps[:],
                            w1_t[:, kc, mc * 128:(mc + 1) * 128],
                            xt["t" if kc < KD else "d"][:, kc % KD, :],
                            start=(kc == 0), stop=(kc == 2 * KD - 1))
                    nc.scalar.activation(
                        ht[:, mc, :], hps[:], AF.Relu,
                        bias=gb1_sb[:, mc:mc + 1])
                lps = psB.tile([1, RC], F32, tag="lps")
                for mc in range(KD):
                    nc.tensor.matmul(
                        lps[:], w2_t[:, mc, :], ht[:, mc, :],
                        start=(mc == 0), stop=(mc == KD - 1))
                nc.scalar.activation(
                    gate_sb[:, rsl], lps[:], AF.Sigmoid, bias=gb2_sb[:])
                nc.vector.tensor_sub(omg_sb[:, rsl], onesrow[:], gate_sb[:, rsl])
                if with_bo:
                    nc.vector.tensor_copy(g2_sb[0:1, rsl], gate_sb[:, rsl])
                    nc.vector.tensor_copy(g2_sb[1:2, rsl], omg_sb[:, rsl])
                # Q projections
                for b in "td":
                    for mc in range(2):
                        qps = psB.tile([128, RC], F32, tag="qps")
                        for kc in range(KD):
                            nc.tensor.matmul(
                                qps[:],
                                wq_t[b][:, kc, mc * 128:(mc + 1) * 128],
                                xt[b][:, kc, :],
                                start=(kc == 0), stop=(kc == KD - 1))
                        nc.scalar.activation(
                            qt_sb[b][:, mc, rsl], qps[:], AF.Identity,
                            bias=bq_sb[:, (0 if b == "t" else 2) + mc:
                                       (0 if b == "t" else 2) + mc + 1])
        # ---- phase C: attention + output projection ----
        # Software pipeline: QK+exp of unit u overlaps PV of unit u-1 at
        # s-chunk granularity; output-projection rows-blocks of the previous
        # rows-chunk interleave between units to keep the PE stream dense.
        with ExitStack() as cctx:
            p_p = cctx.enter_context(tc.tile_pool(name="p_p", bufs=4))
            p_a = cctx.enter_context(tc.tile_pool(name="p_a", bufs=2))
            p_s = cctx.enter_context(tc.tile_pool(name="p_s", bufs=2))
            p_o = cctx.enter_context(tc.tile_pool(name="p_o", bufs=2))
            psS = cctx.enter_context(tc.tile_pool(name="psS", bufs=1, space="PSUM"))
            psPV = cctx.enter_context(tc.tile_pool(name="psPV", bufs=1, space="PSUM"))
            psO = cctx.enter_context(tc.tile_pool(name="psO", bufs=2, space="PSUM"))

            def emit_qk_exp(b, mc, si, rsl):
                sps2 = psS.tile([SCH, 2, RC], F32, tag="sps", name="sps")
                for hh in range(2):  # row-packed pair, adjacent emission
                    po = hh * 64
                    nc.tensor.matmul(
                        sps2[:, hh, :],
                        kt_sb[b][po:po + 64, mc, si * SCH:(si + 1) * SCH],
                        qt_sb[b][po:po + 64, mc, rsl],
                        start=True, stop=True,
                        tile_position=(po, 0))
                p2 = p_p.tile([SCH, 2, RC], BF16, tag=f"p{si % 2}",
                              name=f"p{si % 2}")
                nc.scalar.activation(p2[:], sps2[:], AF.Exp, scale=0.125)
                return p2

            def emit_pv(aps, b, mc, si, p2):
                for hh in range(2):
                    h = mc * 2 + hh
                    nc.tensor.matmul(
                        aps[hh][:], vx_sb[b][:, si, h, :], p2[:, hh, :],
                        start=(si == 0), stop=(si == NSC - 1))

            def emit_norm(aps, b, mc, a2, rsl):
                gcoef = gate_sb if b == "t" else omg_sb
                for hh in range(2):
                    den1 = p_s.tile([1, RC], F32, tag="den1")
                    nc.vector.tensor_copy(den1[:], aps[hh][64:65, :])
                    rec1 = p_s.tile([1, RC], F32, tag="rec1")
                    nc.vector.reciprocal_approx_fast(rec1[:], den1[:])
                    sct1 = p_s.tile([1, RC], F32, tag="sct1")
                    nc.vector.tensor_mul(sct1[:], rec1[:], gcoef[:, rsl])
                    sct64 = p_s.tile([64, RC], F32, tag="sct64")
                    nc.gpsimd.partition_broadcast(sct64[:], sct1[:])
                    nc.vector.tensor_mul(
                        a2[b][mc][hh * 64:hh * 64 + 64, :],
                        aps[hh][0:64, :], sct64[:])

            def emit_outproj_block(a2p, r_prev, rb):
                row0 = r_prev * RC + rb * 128
                for half in range(2):
                    osb = p_o.tile([128, DLLM // 2], F32, tag="osb", name="osb")
                    for nh in range(4):
                        ncc = half * 4 + nh
                        nsl = slice(ncc * 512, (ncc + 1) * 512)
                        ops = psO.tile([128, 512], F32, tag="ops")
                        chains = [("t", 0), ("t", 1), ("d", 0), ("d", 1)]
                        for kk, (bb, mcc) in enumerate(chains):
                            nc.tensor.matmul(
                                ops[:], a2p[bb][mcc][:, rb * 128:(rb + 1) * 128],
                                wo_t[:, kk, nsl],
                                start=(kk == 0), stop=(kk == 3 and not with_bo))
                        if with_bo:
                            nc.tensor.matmul(
                                ops[:], g2_sb[:, row0:row0 + 128], bo_sb[:, nsl],
                                start=False, stop=True)
                        nc.vector.tensor_copy(osb[:, nh * 512:(nh + 1) * 512], ops[:])
                    nc.sync.dma_start(
                        out.ap()[row0:row0 + 128,
                                 half * (DLLM // 2):(half + 1) * (DLLM // 2)], osb[:])

            # phase B work for the last rows-chunk, emitted piecewise between
            # r=0's attention units as dense PE filler
            p_x2 = cctx.enter_context(tc.tile_pool(name="p_x2", bufs=1))
            bstate = {}   # rB -> (xtB dict, lacc)

            def emit_b3_load(rB):
                rslB = slice(rB * RC, (rB + 1) * RC)
                xtB = {}
                for b in "td":
                    xtB[b] = p_x2.tile([128, KD, RC], BF16, tag=f"xB_{b}",
                                       name=f"xB_{b}")
                    nc.sync.dma_start(
                        xtB[b][:],
                        xT[b].ap().rearrange("(c p) n -> p c n", p=128)[:, :, rslB])
                lacc = p_s.tile([1, RC], F32, tag="lacc", name="lacc")
                bstate[rB] = (xtB, lacc)

            def emit_b3_piece(rB, step):
                rslB = slice(rB * RC, (rB + 1) * RC)
                xtB, lacc = bstate[rB]
                # steps 0..7: gate hidden m-chunk (+ its logit partial);
                # 8: sigmoid; 9..12: qT chains
                if step < KD:
                    mc = step
                    hps = psO.tile([128, RC], F32, tag="ops", name="hpsB")
                    for kc in range(2 * KD):
                        nc.tensor.matmul(
                            hps[:],
                            w1_t[:, kc, mc * 128:(mc + 1) * 128],
                            xtB["t" if kc < KD else "d"][:, kc % KD, :],
                            start=(kc == 0), stop=(kc == 2 * KD - 1))
                    htmp = p_s.tile([128, RC], BF16, tag="htmp", name="htmp")
                    nc.scalar.activation(
                        htmp[:], hps[:], AF.Relu, bias=gb1_sb[:, mc:mc + 1])
                    lps = psO.tile([128, RC], F32, tag="ops", name="lpsB")
                    nc.tensor.matmul(lps[0:1, :], w2_t[:, mc, :], htmp[:],
                                     start=True, stop=True)
                    if mc == 0:
                        nc.vector.tensor_copy(lacc[:], lps[0:1, :])
                    else:
                        nc.vector.tensor_add(lacc[:], lacc[:], lps[0:1, :])
                elif step == KD:
                    nc.scalar.activation(
                        gate_sb[:, rslB], lacc[:], AF.Sigmoid, bias=gb2_sb[:])
                    nc.vector.tensor_sub(
                        omg_sb[:, rslB], onesrow[:], gate_sb[:, rslB])
                    if with_bo:
                        nc.vector.tensor_copy(g2_sb[0:1, rslB], gate_sb[:, rslB])
                        nc.vector.tensor_copy(g2_sb[1:2, rslB], omg_sb[:, rslB])
                else:
                    bq, mcq = divmod(step - KD - 1, 2)
                    bb = "t" if bq == 0 else "d"
                    qps = psO.tile([128, RC], F32, tag="ops", name="qpsB")
                    for kc in range(KD):
                        nc.tensor.matmul(
                            qps[:],
                            wq_t[bb][:, kc, mcq * 128:(mcq + 1) * 128],
                            xtB[bb][:, kc, :],
                            start=(kc == 0), stop=(kc == KD - 1))
                    nc.scalar.activation(
                        qt_sb[bb][:, mcq, rslB], qps[:], AF.Identity,
                        bias=bq_sb[:, (0 if bb == "t" else 2) + mcq:
                                   (0 if bb == "t" else 2) + mcq + 1])

            # deferred phase-B work for the last two rows-chunks: 26 pieces
            b3_tasks = []
            for rB in (NRC - 1,):
                b3_tasks.append(lambda rB=rB: emit_b3_load(rB))
                for s in range(KD + 5):
                    b3_tasks.append(lambda rB=rB, s=s: emit_b3_piece(rB, s))

            def make_outproj_tasks(a2p, r_prev):
                """One task per PSUM group; osb assembled per (rb, half)."""
                state = {}

                def group(rb, ncc):
                    row0 = r_prev * RC + rb * 128
                    half, nh = divmod(ncc, 4)
                    if nh == 0:
                        state[(rb, half)] = p_o.tile([128, DLLM // 2], F32,
                                                     tag="osb", name="osb")
                    osb = state[(rb, half)]
                    nsl = slice(ncc * 512, (ncc + 1) * 512)
                    ops = psO.tile([128, 512], F32, tag="ops", name="ops")
                    chains = [("t", 0), ("t", 1), ("d", 0), ("d", 1)]
                    for kk, (bb, mcc) in enumerate(chains):
                        nc.tensor.matmul(
                            ops[:], a2p[bb][mcc][:, rb * 128:(rb + 1) * 128],
                            wo_t[:, kk, nsl],
                            start=(kk == 0), stop=(kk == 3 and not with_bo))
                    if with_bo:
                        nc.tensor.matmul(
                            ops[:], g2_sb[:, row0:row0 + 128], bo_sb[:, nsl],
                            start=False, stop=True)
                    nc.vector.tensor_copy(osb[:, nh * 512:(nh + 1) * 512], ops[:])
                    if nh == 3:
                        nc.sync.dma_start(
                            out.ap()[row0:row0 + 128,
                                     half * (DLLM // 2):(half + 1) * (DLLM // 2)],
                            osb[:])

                return [(lambda rb=rb, ncc=ncc: group(rb, ncc))
                        for rb in range(4) for ncc in range(8)]

            pending = None   # (a2 dict, r) awaiting output projection
            for r in range(NRC):
                rsl = slice(r * RC, (r + 1) * RC)
                a2 = {b: [p_a.tile([128, RC], BF16, tag=f"a2_{b}{mc}",
                                   name=f"a2_{b}{mc}")
                          for mc in range(2)] for b in "td"}
                units = [(b, mc) for b in "td" for mc in range(2)]
                # filler: out-proj groups of r-1 (32 tasks), or the deferred
                # phase-B work for the last rows-chunk during r=0 (13 tasks)
                if pending is not None:
                    tasks = make_outproj_tasks(pending[0], pending[1])
                else:
                    tasks = b3_tasks
                ti = 0
                nslots = len(units) * NSC
                prev = None   # (aps, b, mc, p2dict)
                slot = 0
                for u, (b, mc) in enumerate(units):
                    aps = [psPV.tile([65, RC], F32, tag=f"aps{mc}{hh}",
                                     name=f"aps{mc}{hh}") for hh in range(2)]
                    p2buf = {}
                    for si in range(NSC):
                        p2buf[si] = emit_qk_exp(b, mc, si, rsl)
                        if prev is not None:
                            paps, pb, pmc, pp2 = prev
                            emit_pv(paps, pb, pmc, si, pp2[si])
                        # paced filler: one task per due slot
                        slot += 1
                        due = (slot * len(tasks)) // nslots
                        while ti < due:
                            tasks[ti]()
                            ti += 1
                    if prev is not None:
                        emit_norm(prev[0], prev[1], prev[2], a2, rsl)
                    prev = (aps, b, mc, p2buf)
                # drain last unit of this rows-chunk
                paps, pb, pmc, pp2 = prev
                for si in range(NSC):
                    emit_pv(paps, pb, pmc, si, pp2[si])
                emit_norm(paps, pb, pmc, a2, rsl)
                while ti < len(tasks):
                    tasks[ti]()
                    ti += 1
                pending = (a2, r)
            for task in make_outproj_tasks(pending[0], pending[1]):
                task()
        pre1ctx.close()

    nc.compile()
    return nc


def _prep_inputs(inputs):
    """Host-side shard + transpose. Returns in_maps for 8 cores."""
    f32 = np.float32
    t = {k: np.asarray(v) for k, v in inputs.items()}
    x_full = {"t": t["trend_emb"].reshape(B * L, D).astype(f32),
              "d": t["detail_emb"].reshape(B * L, D).astype(f32)}
    pT_full = {"t": np.ascontiguousarray(t["trend_proto"].astype(f32).T).astype(bf16),
               "d": np.ascontiguousarray(t["detail_proto"].astype(f32).T).astype(bf16)}
    W = {("q", "t"): t["t_Wq"], ("q", "d"): t["d_Wq"],
         ("k", "t"): t["t_Wk"], ("k", "d"): t["d_Wk"],
         ("v", "t"): t["t_Wv"], ("v", "d"): t["d_Wv"],
         ("o", "t"): t["t_Wo"], ("o", "d"): t["d_Wo"]}
    bias = {("q", "t"): t["t_bq"], ("q", "d"): t["d_bq"],
            ("k", "t"): t["t_bk"], ("k", "d"): t["d_bk"],
            ("v", "t"): t["t_bv"], ("v", "d"): t["d_bv"],
            ("o", "t"): t["t_bo"], ("o", "d"): t["d_bo"]}

    with_bo = bool(np.any(bias[("o", "t")]) or np.any(bias[("o", "d")]))
    in_maps = []
    for core in range(8):
        rg, hg = divmod(core, HG)
        rows = slice(rg * R, (rg + 1) * R)
        hsl = slice(hg * HEC, (hg + 1) * HEC)
        m = {}
        for b in "td":
            m[f"xT_{b}"] = np.ascontiguousarray(x_full[b][rows].T).astype(bf16)
            m[f"pT_{b}"] = pT_full[b]
            m[f"wq_{b}"] = np.ascontiguousarray(W[("q", b)][:, hsl]).astype(bf16)
            m[f"wk_{b}"] = np.ascontiguousarray(W[("k", b)][:, hsl]).astype(bf16)
            m[f"wv_{b}"] = np.ascontiguousarray(W[("v", b)][:, hsl]).astype(bf16)
        m["wo"] = np.vstack([W[("o", "t")][hsl, :], W[("o", "d")][hsl, :]]).astype(bf16)
        m["w1"] = t["g_W1"].astype(bf16)
        m["w2"] = t["g_W2"].astype(bf16)
        m["bq2"] = np.stack([bias[("q", "t")][hsl][0:128], bias[("q", "t")][hsl][128:256],
                             bias[("q", "d")][hsl][0:128], bias[("q", "d")][hsl][128:256]],
                            axis=1).astype(f32)
        m["bk2"] = np.stack([bias[("k", "t")][hsl][0:128], bias[("k", "t")][hsl][128:256],
                             bias[("k", "d")][hsl][0:128], bias[("k", "d")][hsl][128:256]],
                            axis=1).astype(f32)
        m["bv"] = np.concatenate([bias[("v", "t")][hsl],
                                  bias[("v", "d")][hsl]])[None, :].astype(bf16)
        m["gb1"] = np.ascontiguousarray(
            t["g_b1"].astype(f32).reshape(KD, 128).T)
        m["gb2"] = t["g_b2"].astype(f32).reshape(1, 1)
        m["ones"] = np.ones((1, 2048), f32)
        if with_bo:
            m["bo2"] = (np.stack([bias[("o", "t")], bias[("o", "d")]]) / HG).astype(bf16)
        in_maps.append(m)
    return in_maps, with_bo


def kernel(**inputs):
    global LAST_RESULTS
    import os
    from concourse.bass_utils import run_bass_kernel_spmd

    in_maps, with_bo = _prep_inputs(inputs)
    if with_bo not in _CACHE:
        _CACHE[with_bo] = _build(with_bo)
    nc = _CACHE[with_bo]

    trace = bool(os.environ.get("KERNEL_TRACE"))
    res = run_bass_kernel_spmd(
        nc, in_maps, list(range(8)),
        trace=trace, trace_cores=list(range(8)) if trace else None)
    LAST_RESULTS = res

    out = np.empty((RG, R, DLLM), np.float32)
    for rg in range(RG):
        acc = res.results[rg * HG]["out"].astype(np.float32)
        for hg in range(1, HG):
            acc = acc + res.results[rg * HG + hg]["out"]
        out[rg] = acc
    return out.reshape(B, L, DLLM)
